# revision 19
# baseline (speedup 1.0000x reference)
"""CrossModalAttention kernel for 8x TRN2 NeuronCores (batch data-parallel).

Per core (one batch element), all operands fp16 (PE rate = fp32r rate, half
the DMA bytes, DVE 2x copy modes):
  Preamble: qkT[0], qkT[6] (pair-0 q/k features, transposed layout) and
    v_aug[0..7] ([token, head*65] with a ones column per head so attn@v
    also yields softmax denominators).
  Stage 2 (heads sequential, ACT-bound pipeline): per head h, per kc
    (8 token chunks of 128): scoresT[k, q] matmuls (K=64) -> one FD=1024
    exp on ACT (scalar engine) -> attn@v accumulated into a single-buffered
    [65, 1024] psum, which is immediately copied raw to SBUF to free the
    bank. Between sc and av matmuls the PE runs "filler": the deferred
    qkT[m] chains for later pairs (keeps PE busy/warm while ACT paces).
    Normalization: row 64 = sums -> reciprocal -> DRAM-broadcast -> mult.
  Stage 3: proj from outT tiles (pair-major rows), bias-free fast path.

PSUM budget: sc 2x[128,1024]f32 (4 banks) + av 1x[65,1024]f32 (2 banks)
+ filler 2x[128,512]f32 (2 banks) = 8 banks.
"""
import numpy as np

import concourse.bass as bass
import concourse.tile as tile
from concourse import bacc, mybir
from concourse.bass_utils import run_bass_kernel_spmd

NP_BF16 = mybir.dt.np(mybir.dt.bfloat16)

DIM = 768
NUM_HEADS = 12
HEAD_DIM = 64
B, N = 8, 1024
P = 128
KC = DIM // P          # 6 contraction chunks of 128 over channels
TC = N // P            # 8 token chunks of 128
HP = NUM_HEADS // 2    # 6 head pairs
VAUG = 65              # v columns per head: 64 v dims + 1 ones column

F32 = mybir.dt.float32
BF16 = mybir.dt.bfloat16
FP8 = mybir.dt.float8e4
VPAD = 80              # padded per-(head, kc-parity) v block in fp8 layout


def build_nc(with_qkv_bias: bool, with_proj_bias: bool):
    nc = bacc.Bacc("TRN2", target_bir_lowering=False, debug=False)

    xT_d = nc.dram_tensor("xT", [DIM, N], BF16, kind="ExternalInput")
    # wqk blocks: row m*P + p, col c*P + j  <=>  W[c*P+p, m*P+j] (q pre-scaled)
    wqkb_d = nc.dram_tensor("wqkb", [2 * DIM, KC * P], BF16, kind="ExternalInput")
    wv_d = nc.dram_tensor("wv", [DIM, DIM], BF16, kind="ExternalInput")
    wproj_d = nc.dram_tensor("wproj", [DIM, DIM], BF16, kind="ExternalInput")
    bqk_d = nc.dram_tensor("bqk", [1, 2 * DIM], BF16, kind="ExternalInput")
    bv_d = nc.dram_tensor("bv", [1, DIM], BF16, kind="ExternalInput")
    bproj_d = nc.dram_tensor("bproj", [1, DIM], BF16, kind="ExternalInput")
    out_d = nc.dram_tensor("out", [N, DIM], BF16, kind="ExternalOutput")

    with tile.TileContext(nc) as tc:
        with (
            tc.tile_pool(name="xw_sb", bufs=1) as xw_pool,
            tc.tile_pool(name="qk_sb", bufs=1) as qk_pool,
            tc.tile_pool(name="vaug_sb", bufs=1) as vaug_pool,
            tc.tile_pool(name="consts", bufs=1) as consts,
            tc.tile_pool(name="expT", bufs=4) as exp_pool,
            tc.tile_pool(name="raw_sb", bufs=3) as raw_pool,
            tc.tile_pool(name="out_sb", bufs=1) as out_pool,
        ):
            # ---- inputs: DMA in priority order ----
            xT = [xw_pool.tile([P, N], BF16, name=f"xT{c}") for c in range(KC)]
            wqkm = [
                xw_pool.tile([P, KC * P], BF16, name=f"wqkm{m}")
                for m in range(2 * KC)
            ]
            wv = [xw_pool.tile([P, DIM], BF16, name=f"wv{c}") for c in range(KC)]
            wproj = [
                xw_pool.tile([P, DIM], BF16, name=f"wproj{c}") for c in range(KC)
            ]
            def dma_wqk_block(m):
                nc.sync.dma_start(
                    out=wqkm[m][:], in_=wqkb_d[m * P : (m + 1) * P, :]
                )

            # pair-0 weights first so preamble matmuls start ASAP
            dma_wqk_block(0)
            dma_wqk_block(6)
            for c in range(KC):
                nc.sync.dma_start(out=xT[c][:], in_=xT_d[c * P : (c + 1) * P, :])
            for c in range(KC):
                nc.sync.dma_start(
                    out=wv[c][:], in_=wv_d[c * P : (c + 1) * P, :]
                )
            for p in range(1, HP):
                dma_wqk_block(p)
                dma_wqk_block(6 + p)
            for c in range(KC):
                nc.sync.dma_start(
                    out=wproj[c][:], in_=wproj_d[c * P : (c + 1) * P, :]
                )

            if with_qkv_bias or with_proj_bias:
                ones_row = consts.tile([1, N], BF16)
                nc.vector.memset(ones_row[:], 1.0)
                bqk_sb = consts.tile([1, 2 * DIM], BF16)
                nc.sync.dma_start(out=bqk_sb[:], in_=bqk_d[:])
                bv_sb = consts.tile([1, DIM], BF16)
                nc.sync.dma_start(out=bv_sb[:], in_=bv_d[:])
                bproj_sb = consts.tile([1, DIM], BF16)
                nc.sync.dma_start(out=bproj_sb[:], in_=bproj_d[:])

            # ---- persistent compute tiles ----
            qkT = [qk_pool.tile([P, N], BF16, name=f"qkT{m}") for m in range(2 * KC)]
            v_aug = [
                vaug_pool.tile([P, NUM_HEADS * VAUG], BF16, name=f"vaug{t}")
                for t in range(TC)
            ]
            outT = [out_pool.tile([P, N], BF16, name=f"outT{p}") for p in range(HP)]

            def emit_qk_chunk(ps_ap, m, qsl):
                """Accumulate qkT[m][:, qsl] into psum AP ps_ap."""
                for c in range(KC):
                    nc.tensor.matmul(
                        ps_ap,
                        wqkm[m][:, c * P : (c + 1) * P],
                        xT[c][:, qsl],
                        start=(c == 0),
                        stop=(c == KC - 1) and not with_qkv_bias,
                    )
                if with_qkv_bias:
                    nc.tensor.matmul(
                        ps_ap,
                        bqk_sb[:, m * P : (m + 1) * P],
                        ones_row[:, qsl],
                        start=False,
                        stop=True,
                    )

            def emit_v_chunk(ps_ap, t, nsl):
                """Accumulate v[t][:, nsl] (natural layout) into psum AP."""
                tsl = slice(t * P, (t + 1) * P)
                for c in range(KC):
                    nc.tensor.matmul(
                        ps_ap,
                        xT[c][:, tsl],
                        wv[c][:, nsl],
                        start=(c == 0),
                        stop=(c == KC - 1) and not with_qkv_bias,
                    )
                if with_qkv_bias:
                    nc.tensor.matmul(
                        ps_ap,
                        ones_row[:, tsl],
                        bv_sb[:, nsl],
                        start=False,
                        stop=True,
                    )

            # ================= preamble =================
            with tc.tile_pool(name="pre_ps", bufs=2, space="PSUM") as pre_ps:
                for m in (0, 6):
                    ps = pre_ps.tile([P, N], F32, name=f"pre_qk{m}", tag="pre")
                    for q in range(2):
                        emit_qk_chunk(ps[:, q * 512 : (q + 1) * 512], m, slice(q * 512, (q + 1) * 512))
                    nc.vector.tensor_copy(qkT[m][:], ps[:])
                for t in range(TC):
                    ps = pre_ps.tile([P, DIM], F32, name=f"pre_v{t}", tag="pre")
                    for nsl in (slice(0, 512), slice(512, DIM)):
                        emit_v_chunk(ps[:, nsl], t, nsl)
                    va3 = v_aug[t][:].rearrange("p (h e) -> p h e", e=VAUG)
                    nc.vector.memset(va3[:, :, 64:65], 1.0)
                    nc.vector.tensor_copy(
                        va3[:, :, 0:64],
                        ps[:].rearrange("p (h d) -> p h d", d=HEAD_DIM),
                    )

            # ================= stage 2 =================
            # Filler: deferred qkT chains for pairs 1..5, emitted one matmul
            # at a time between stage-2 ops to fill PE slack under ACT.
            filler_jobs = []  # list of (m, qhalf)
            for p in range(1, HP):
                for m in (p, 6 + p):
                    for qh in range(2):
                        filler_jobs.append((m, qh))

            with (
                tc.tile_pool(name="norm", bufs=2) as norm_pool,
                tc.tile_pool(name="rep", bufs=2) as rep_pool,
                tc.tile_pool(name="dramp", bufs=1, space="DRAM") as dram_pool,
                tc.tile_pool(name="ps_sc", bufs=2, space="PSUM") as ps_sc,
                tc.tile_pool(name="ps_av", bufs=1, space="PSUM") as ps_av,
                tc.tile_pool(name="ps_fill", bufs=2, space="PSUM") as ps_fill,
            ):
                recip_d = dram_pool.tile([NUM_HEADS, N], BF16)

                fill_state = {"ps": None, "c": 0, "job": None, "idx": 0}

                def emit_filler(n_mms):
                    """Emit up to n_mms filler matmuls (deferred qkT work)."""
                    for _ in range(n_mms):
                        st = fill_state
                        if st["job"] is None:
                            if st["idx"] >= len(filler_jobs):
                                return
                            st["job"] = filler_jobs[st["idx"]]
                            st["idx"] += 1
                            st["c"] = 0
                            m, qh = st["job"]
                            st["ps"] = ps_fill.tile(
                                [P, 512], F32, name=f"fl{m}_{qh}", tag="fl"
                            )
                        m, qh = st["job"]
                        c = st["c"]
                        qsl = slice(qh * 512, (qh + 1) * 512)
                        last = (c == KC - 1) and not with_qkv_bias
                        nc.tensor.matmul(
                            st["ps"][:],
                            wqkm[m][:, c * P : (c + 1) * P],
                            xT[c][:, qsl],
                            start=(c == 0),
                            stop=last,
                        )
                        st["c"] += 1
                        if st["c"] == KC:
                            if with_qkv_bias:
                                nc.tensor.matmul(
                                    st["ps"][:],
                                    bqk_sb[:, m * P : (m + 1) * P],
                                    ones_row[:, qsl],
                                    start=False,
                                    stop=True,
                                )
                            nc.vector.tensor_copy(qkT[m][:, qsl], st["ps"][:])
                            st["job"] = None
                            st["ps"] = None

                # Global software pipeline over steps (h, kc): sc+exp for
                # step s, then av (DoubleRow fp8, one kc-PAIR at a time)
                # for the step-(s-1) pair; lag crosses head boundaries so
                # ACT never waits at them.
                steps = [(h, kc) for h in range(NUM_HEADS) for kc in range(TC)]
                av_tile = [None]
                eT_hist = {}

                def emit_av(h, kc):
                    if kc == 0:
                        av_tile[0] = ps_av.tile(
                            [VAUG, N], F32, name=f"av{h}", tag="av"
                        )
                    av = av_tile[0]
                    eT = eT_hist.pop((h, kc))
                    for q in range(2):
                        qsl = slice(q * 512, (q + 1) * 512)
                        nc.tensor.matmul(
                            av[:, qsl],
                            v_aug[kc][:, h * VAUG : (h + 1) * VAUG],
                            eT[:, qsl],
                            start=(kc == 0),
                            stop=(kc == TC - 1),
                        )
                    if kc == TC - 1:
                        finalize_head(h, av)

                def finalize_head(h, av):
                    p, half = h // 2, h % 2
                    rsl = slice(64 * half, 64 * half + 64)
                    # free the av psum bank ASAP: sums row (fp32) + raw copy
                    sums_t = norm_pool.tile([1, N], F32, name=f"sums{h}", tag="sums")
                    nc.vector.tensor_copy(sums_t[:], av[64:65, :])
                    raw = raw_pool.tile([VAUG, N], BF16, name=f"raw{h}", tag="raw")
                    nc.vector.tensor_copy(raw[:], av[:])
                    # denominators -> reciprocal -> broadcast -> normalize
                    recip_t = norm_pool.tile([1, N], F32, name=f"recip{h}", tag="recip")
                    nc.vector.reciprocal_approx_fast(out=recip_t[:], in_=sums_t[:])
                    recip_b = norm_pool.tile([1, N], BF16, name=f"recipb{h}", tag="recipb")
                    nc.vector.tensor_copy(recip_b[:], recip_t[:])
                    nc.sync.dma_start(out=recip_d[h : h + 1, :], in_=recip_b[:])
                    rep = rep_pool.tile([64, N], BF16, name=f"rep{h}", tag="rep")
                    nc.sync.dma_start(
                        out=rep[:],
                        in_=recip_d[h : h + 1, :].to_broadcast([64, N]),
                    )
                    nc.vector.tensor_tensor(
                        out=outT[p][rsl, :],
                        in0=raw[0:64, :],
                        in1=rep[:],
                        op=mybir.AluOpType.mult,
                    )

                for s, (h, kc) in enumerate(steps):
                    p, half = h // 2, h % 2
                    rsl = slice(64 * half, 64 * half + 64)
                    qT = qkT[p]
                    kT = qkT[KC + p]
                    sc = ps_sc.tile([P, N], F32, name=f"sc{h}_{kc}", tag="sc")
                    ksl = slice(kc * P, (kc + 1) * P)
                    for q in range(2):
                        qsl = slice(q * 512, (q + 1) * 512)
                        nc.tensor.matmul(
                            sc[:, qsl], kT[rsl, ksl], qT[rsl, qsl],
                            start=True, stop=True,
                        )
                    eT = exp_pool.tile([P, N], BF16, name=f"e{h}_{kc}", tag="e")
                    eT_hist[(h, kc)] = eT
                    nc.scalar.activation(
                        eT[:], sc[:], mybir.ActivationFunctionType.Exp
                    )
                    emit_filler(2 if s % 2 == 0 else 1)
                    if s > 1:
                        emit_av(*steps[s - 2])
                for hk in (steps[-2], steps[-1]):
                    emit_av(*hk)

            # ================= stage 3: proj =================
            with (
                tc.tile_pool(name="ps_pj", bufs=3, space="PSUM") as ps_pj,
                tc.tile_pool(name="fin", bufs=3) as fin_pool,
            ):
                for t in range(TC):
                    ps = ps_pj.tile([P, DIM], F32, name=f"pj{t}", tag="pj")
                    tsl = slice(t * P, (t + 1) * P)
                    for c in range(KC):
                        for nsl in (slice(0, 512), slice(512, DIM)):
                            nc.tensor.matmul(
                                ps[:, nsl],
                                outT[c][:, tsl],
                                wproj[c][:, nsl],
                                start=(c == 0),
                                stop=(c == KC - 1) and not with_proj_bias,
                            )
                    if with_proj_bias:
                        for nsl in (slice(0, 512), slice(512, DIM)):
                            nc.tensor.matmul(
                                ps[:, nsl],
                                ones_row[:, t * P : t * P + P],
                                bproj_sb[:, nsl],
                                start=False,
                                stop=True,
                            )
                    fin = fin_pool.tile([P, DIM], BF16, name=f"fin{t}", tag="fin")
                    nc.vector.tensor_copy(fin[:], ps[:])
                    nc.sync.dma_start(out=out_d[tsl, :], in_=fin[:])

    nc.compile()
    return nc


_NC_CACHE = {}


def _prep_weights(qkv_w, qkv_b, proj_w, proj_b):
    scale = HEAD_DIM ** -0.5
    wqk = qkv_w[:, : 2 * DIM].copy()
    wqk[:, :DIM] *= scale
    # blocks: wqkb[m*P + p, c*P + j] = wqk[c*P + p, m*P + j]
    wqkb = np.ascontiguousarray(
        wqk.reshape(KC, P, 2 * KC, P)
        .transpose(2, 1, 0, 3)
        .reshape(2 * DIM, KC * P)
    ).astype(NP_BF16)
    wv = np.ascontiguousarray(qkv_w[:, 2 * DIM :]).astype(NP_BF16)
    bqk = qkv_b[: 2 * DIM].copy()
    bqk[:DIM] *= scale
    return {
        "wqkb": wqkb,
        "wv": wv,
        "wproj": proj_w.astype(NP_BF16),
        "bqk": bqk.reshape(1, -1).astype(NP_BF16),
        "bv": qkv_b[2 * DIM :].reshape(1, -1).astype(NP_BF16),
        "bproj": proj_b.reshape(1, -1).astype(NP_BF16),
    }


def kernel(**inputs) -> np.ndarray:
    x = np.asarray(inputs["x"], dtype=np.float32)
    qkv_w = np.asarray(inputs["qkv_w"], dtype=np.float32)
    qkv_b = np.asarray(inputs["qkv_b"], dtype=np.float32)
    proj_w = np.asarray(inputs["proj_w"], dtype=np.float32)
    proj_b = np.asarray(inputs["proj_b"], dtype=np.float32)
    # context is unused by the reference layer.

    with_qkv_bias = bool(np.any(qkv_b))
    with_proj_bias = bool(np.any(proj_b))

    key = (with_qkv_bias, with_proj_bias)
    if key not in _NC_CACHE:
        _NC_CACHE[key] = build_nc(*key)
    nc = _NC_CACHE[key]

    base = _prep_weights(qkv_w, qkv_b, proj_w, proj_b)
    in_maps = [
        {**base, "xT": np.ascontiguousarray(x[b].T).astype(NP_BF16)}
        for b in range(B)
    ]
    res = run_bass_kernel_spmd(nc, in_maps, list(range(B)))
    out = np.stack([res.results[b]["out"] for b in range(B)], axis=0)
    return out.astype(np.float32)
